# revision 8
# baseline (speedup 1.0000x reference)
# Trainium2 Bass kernel for nn_Mixtral_72851235275310 (2-block Mixtral-style
# transformer: sliding-window attention + top-2-of-8 MoE + LM head).
#
# Sharding over 8 NeuronCores:
#   - attention: head-parallel (2 of 16 heads per core) + AllGather of o
#   - MoE: expert-parallel (1 expert per core, dense gated compute) + AllReduce
#   - LM head: vocab-parallel (4000 cols per core), host-side concat
#
# Precision: the residual stream / attention / router run in plain fp32 PE
# matmuls (router top-2 selection must match the fp32 reference exactly);
# the expert FFNs and LM head run in fp16 (fp32 accumulate in PSUM).
# RMSNorm scale g and final_rms_scale are folded into consumer weights /
# rope tables on the host (exact for arbitrary g).
import os
import sys

sys.path.insert(0, "/opt/trn_rl_repo")

import numpy as np

# ---- model dims (hardcoded per spec) ----
B, T, D, H = 1, 1024, 1024, 16
HD = D // H              # 64
E, TOPK, NB = 8, 2, 2
V = 32000
VOUT = V - 1
FF = 4 * D               # 4096
EPS = 1e-6
ROPE_BASE = 10000.0
NCORE = 8
HPC = H // NCORE         # 2 heads per core
WIN = T // 2             # 512-token sliding window
BAND = WIN + 128         # 640 key->query band per key tile
VSH = V // NCORE         # 4000 (padded vocab shard)
KT = D // 128            # 8 k-tiles over D
FFT = FF // 128          # 32 tiles over FF
NT = T // 128            # 8 token tiles

F32 = "float32"
F16 = "float16"
I32 = "int32"

_CACHE = {}


# ---------------------------------------------------------------------------
# host-side constant builders
# ---------------------------------------------------------------------------
def _make_rope_tables(g):
    # feature-major [D, T] cos/sin with per-d g folded in.
    pos = np.arange(T, dtype=np.float64)
    theta = 1.0 / (ROPE_BASE ** (np.arange(0, HD, 2, dtype=np.float64) / HD))
    ang = pos[:, None] * theta[None, :]          # [T, 32]
    c = np.cos(ang)                               # [T, 32]
    s = np.sin(ang)
    i_of_d = (np.arange(D) % HD) // 2             # pair index per d
    partner = np.arange(D) + np.where(np.arange(D) % 2 == 0, 1, -1)
    cT = (c[:, i_of_d].T * g[:, None]).astype(np.float32)        # [D, T]
    sT = (s[:, i_of_d].T * g[partner][:, None]).astype(np.float32)
    return cT, sT


def _make_perm():
    # xshift = Perm @ x : xshift[2i] = -x[2i+1], xshift[2i+1] = x[2i]
    # matmul computes lhsT.T @ rhs, so pass PermT = Perm.T
    P = np.zeros((128, 128), np.float32)
    idx = np.arange(64)
    P[2 * idx, 2 * idx + 1] = -1.0
    P[2 * idx + 1, 2 * idx] = 1.0
    return P.T.copy()


def _make_maskT():
    # [128, 640] additive pre-scale mask: row k_local, col q_rel
    # (q = ki*128 + q_rel); allowed iff k_local <= q_rel < k_local + WIN
    k = np.arange(128)[:, None]
    q = np.arange(BAND)[None, :]
    allowed = (q >= k) & (q < k + WIN)
    return np.where(allowed, 0.0, -1e31).astype(np.float32)


# ---------------------------------------------------------------------------
# device program
# ---------------------------------------------------------------------------
def _build_nc():
    import concourse.bass as bass
    import concourse.tile as tile
    from concourse import bacc, mybir

    dt = mybir.dt
    AF = mybir.ActivationFunctionType
    OP = mybir.AluOpType
    AX = mybir.AxisListType

    nc = bacc.Bacc("TRN2", target_bir_lowering=False, debug=False,
                   num_devices=NCORE)

    def din(name, shape, d=dt.float32):
        return nc.dram_tensor(name, list(shape), d, kind="ExternalInput").ap()

    # ---- external inputs (identical shapes on all cores; data differs) ----
    idx_i = din("idx_i", [T, 1], dt.int32)
    emb = din("emb", [V, D])
    ident = din("ident", [128, 128])
    onesm = din("onesm", [128, 128])
    permT = din("permT", [128, 128])
    maskT_d = din("maskT", [128, BAND])
    sel_bc = din("sel_bc", [128, E])
    cosT = din("cosT", [NB, D, T])
    sinT = din("sinT", [NB, D, T])
    wqT = din("wqT", [NB, D, 128])
    wkT = din("wkT", [NB, D, 128])
    wvT = din("wvT", [NB, D, 128])
    bq_d = din("bq", [NB, 128, 1])
    bk_d = din("bk", [NB, 128, 1])
    bv_d = din("bv_bc", [NB, 128, 128])
    woT = din("woT", [NB, D, D])
    wob_d = din("wob", [NB, 128, KT])
    rtT = din("rtT", [NB, D, E])
    rb_d = din("rb_bc", [NB, 128, E])
    w1T = din("w1T", [NB, D, FF], dt.float16)
    b1_d = din("b1c", [NB, 128, FFT])
    w2T = din("w2T", [NB, FF, D], dt.float16)
    b2_d = din("b2c", [NB, 128, KT])
    lmT = din("lmT", [D, VSH], dt.float16)
    lmb_d = din("lmb_bc", [128, VSH])

    logits = nc.dram_tensor("logits", [T, VSH], dt.float32,
                            kind="ExternalOutput").ap()

    RG = [list(range(NCORE))]

    with tile.TileContext(nc, num_cores=NCORE) as tc:
        gconst = tc.alloc_tile_pool(name="gconst", bufs=1)
        resid = tc.alloc_tile_pool(name="resid", bufs=1)
        dram = tc.alloc_tile_pool(name="dram", bufs=1, space="DRAM")

        ident_sb = gconst.tile([128, 128], dt.float32)
        ones_sb = gconst.tile([128, 128], dt.float32)
        perm_sb = gconst.tile([128, 128], dt.float32)
        mask_sb = gconst.tile([128, BAND], dt.float32)
        sel_sb = gconst.tile([128, E], dt.float32)
        eps_sb = gconst.tile([128, 1], dt.float32)
        nc.gpsimd.memset(eps_sb[:], EPS)
        nc.gpsimd.dma_start(ident_sb[:], ident)
        nc.gpsimd.dma_start(ones_sb[:], onesm)
        nc.gpsimd.dma_start(perm_sb[:], permT)
        nc.gpsimd.dma_start(mask_sb[:], maskT_d)
        nc.gpsimd.dma_start(sel_sb[:], sel_bc)

        # ---------------- embedding gather + transpose -> xT [128, KT, T] ---
        xT = resid.tile([128, KT, T], dt.float32, tag="x")
        with tc.tile_pool(name="gath", bufs=2) as gp, \
             tc.tile_pool(name="gps", bufs=2, space="PSUM") as gps:
            for ti in range(NT):
                idx_sb = gp.tile([128, 1], dt.int32, tag="idx")
                nc.sync.dma_start(idx_sb[:], idx_i[ti * 128:(ti + 1) * 128, :])
                xg = gp.tile([128, D], dt.float32, tag="xg")
                nc.gpsimd.indirect_dma_start(
                    out=xg[:], out_offset=None, in_=emb,
                    in_offset=bass.IndirectOffsetOnAxis(ap=idx_sb[:, :1], axis=0),
                )
                for m in range(KT):
                    tp = gps.tile([128, 128], dt.float32, tag="tp")
                    nc.tensor.transpose(tp[:], xg[:, m * 128:(m + 1) * 128],
                                        ident_sb[:])
                    eng = nc.scalar if (m % 2 == 0) else nc.vector
                    if m % 2 == 0:
                        nc.scalar.copy(xT[:, m, ti * 128:(ti + 1) * 128], tp[:])
                    else:
                        nc.vector.tensor_copy(xT[:, m, ti * 128:(ti + 1) * 128],
                                              tp[:])

        # ------------- rmsnorm helper (feature-major; g folded elsewhere) ---
        def rmsnorm(x_in, pool, psp, out_f32=None, out_f16=None):
            # sum(x^2) over D via all-ones matmul -> broadcast across parts
            var_ps = psp.tile([128, T], dt.float32, tag="var")
            for k in range(KT):
                sq = pool.tile([128, T], dt.float32, tag="sqk", bufs=2)
                nc.vector.tensor_mul(sq[:], x_in[:, k, :], x_in[:, k, :])
                for nh in range(2):
                    nc.tensor.matmul(
                        var_ps[:, nh * 512:(nh + 1) * 512], ones_sb[:],
                        sq[:, nh * 512:(nh + 1) * 512],
                        start=(k == 0), stop=(k == KT - 1))
            rstd = pool.tile([128, T], dt.float32, tag="rstd", bufs=1)
            # sqrt(var/D + eps)
            nc.scalar.activation(rstd[:], var_ps[:], AF.Sqrt,
                                 bias=eps_sb[:, :1], scale=1.0 / D)
            rinv = pool.tile([128, T], dt.float32, tag="rinv", bufs=1)
            nc.vector.reciprocal(rinv[:], rstd[:])
            for k in range(KT):
                if out_f32 is not None:
                    nc.vector.tensor_mul(out_f32[:, k, :], x_in[:, k, :],
                                         rinv[:])
                    if out_f16 is not None:
                        nc.scalar.copy(out_f16[:, k, :], out_f32[:, k, :])
                elif out_f16 is not None:
                    nc.vector.tensor_mul(out_f16[:, k, :], x_in[:, k, :],
                                         rinv[:])

        # =================== transformer blocks ===========================
        for blk in range(NB):
            # ---------------- attention -----------------------------------
            qkv_out = tc.alloc_tile_pool(name=f"qkvo{blk}", bufs=1)
            qpT = qkv_out.tile([128, T], dt.float32, tag="qpT")
            kpT = qkv_out.tile([128, T], dt.float32, tag="kpT")
            vp = qkv_out.tile([128, NT, 128], dt.float32, tag="vp")
            o2T = qkv_out.tile([128, T], dt.float32, tag="o2T")

            with tc.tile_pool(name=f"attA{blk}", bufs=2) as pa, \
                 tc.tile_pool(name=f"psA{blk}", bufs=1, space="PSUM") as psa:
                xn = pa.tile([128, KT, T], dt.float32, tag="xn", bufs=1)
                rmsnorm(xT, pa, psa, out_f32=xn)

                wq_sb = pa.tile([128, KT, 128], dt.float32, tag="wq", bufs=1)
                wk_sb = pa.tile([128, KT, 128], dt.float32, tag="wk", bufs=1)
                wv_sb = pa.tile([128, KT, 128], dt.float32, tag="wv", bufs=1)
                for k in range(KT):
                    sl = slice(k * 128, (k + 1) * 128)
                    nc.sync.dma_start(wq_sb[:, k], wqT[blk, sl])
                    nc.sync.dma_start(wk_sb[:, k], wkT[blk, sl])
                    nc.sync.dma_start(wv_sb[:, k], wvT[blk, sl])
                bqs = pa.tile([128, 1], dt.float32, tag="bqs")
                bks = pa.tile([128, 1], dt.float32, tag="bks")
                bvs = pa.tile([128, 128], dt.float32, tag="bvs")
                nc.sync.dma_start(bqs[:], bq_d[blk])
                nc.sync.dma_start(bks[:], bk_d[blk])
                nc.sync.dma_start(bvs[:], bv_d[blk])

                qp_ps = psa.tile([128, T], dt.float32, tag="qp")
                kp_ps = psa.tile([128, T], dt.float32, tag="kp")
                for k in range(KT):
                    # rope_k = xn_k * cos_k + (Perm @ xn_k) * sin_k
                    sh_ps = psa.tile([128, T], dt.float32, tag="var")
                    for nh in range(2):
                        nc.tensor.matmul(sh_ps[:, nh * 512:(nh + 1) * 512],
                                         perm_sb[:],
                                         xn[:, k, nh * 512:(nh + 1) * 512],
                                         start=True, stop=True)
                    csl = slice(k * 128, (k + 1) * 128)
                    ck = pa.tile([128, T], dt.float32, tag="ck")
                    sk = pa.tile([128, T], dt.float32, tag="sk")
                    nc.sync.dma_start(ck[:], cosT[blk, csl])
                    nc.sync.dma_start(sk[:], sinT[blk, csl])
                    t1 = pa.tile([128, T], dt.float32, tag="t1")
                    t2 = pa.tile([128, T], dt.float32, tag="t2")
                    nc.vector.tensor_mul(t1[:], xn[:, k, :], ck[:])
                    nc.vector.tensor_mul(t2[:], sh_ps[:], sk[:])
                    rk = pa.tile([128, T], dt.float32, tag="rk")
                    nc.vector.tensor_add(rk[:], t1[:], t2[:])
                    for nh in range(2):
                        nsl = slice(nh * 512, (nh + 1) * 512)
                        nc.tensor.matmul(qp_ps[:, nsl], wq_sb[:, k], rk[:, nsl],
                                         start=(k == 0), stop=(k == KT - 1))
                        nc.tensor.matmul(kp_ps[:, nsl], wk_sb[:, k], rk[:, nsl],
                                         start=(k == 0), stop=(k == KT - 1))
                # drains (+bias)
                for nh in range(2):
                    nsl = slice(nh * 512, (nh + 1) * 512)
                    nc.scalar.activation(qpT[:, nsl], qp_ps[:, nsl],
                                         AF.Identity, bias=bqs[:, :1])
                    nc.scalar.activation(kpT[:, nsl], kp_ps[:, nsl],
                                         AF.Identity, bias=bks[:, :1])
                # vp token-major [t, 2 heads x 64]
                for m in range(NT):
                    vp_ps = psa.tile([128, 128], dt.float32, tag="vpp")
                    for k in range(KT):
                        nc.tensor.matmul(vp_ps[:],
                                         xn[:, k, m * 128:(m + 1) * 128],
                                         wv_sb[:, k],
                                         start=(k == 0), stop=(k == KT - 1))
                    nc.vector.tensor_add(vp[:, m, :], vp_ps[:], bvs[:])

            # ----- scores -> exp -> o (banded sliding window) --------------
            with tc.tile_pool(name=f"attB{blk}", bufs=1) as pb, \
                 tc.tile_pool(name=f"psB{blk}", bufs=1, space="PSUM") as psb:
                expt = {}
                for ki in range(NT):
                    band = min(BAND, T - ki * 128)
                    for h in range(HPC):
                        hsl = slice(h * 64, (h + 1) * 64)
                        sc_ps = psb.tile([128, BAND], dt.float32,
                                         tag=f"sc{h}")
                        for c0 in range(0, band, 512):
                            n = min(512, band - c0)
                            cs = slice(c0, c0 + n)
                            qs = slice(ki * 128 + c0, ki * 128 + c0 + n)
                            nc.tensor.matmul(sc_ps[:, cs],
                                             kpT[hsl, ki * 128:(ki + 1) * 128],
                                             qpT[hsl, qs],
                                             start=True, stop=False)
                            nc.tensor.matmul(sc_ps[:, cs], ident_sb[:],
                                             mask_sb[:, cs],
                                             start=False, stop=True)
                        ex = pb.tile([128, BAND], dt.float32,
                                     tag=f"exp_{ki}_{h}")
                        nc.scalar.activation(ex[:, :band], sc_ps[:, :band],
                                             AF.Exp, scale=0.125)
                        expt[(ki, h)] = ex
                for qi in range(NT):
                    kis = list(range(max(0, qi - 4), qi + 1))
                    o_ps = psb.tile([128, 128], dt.float32, tag="o")
                    dn_ps = psb.tile([128, 128], dt.float32, tag="dn")
                    for h in range(HPC):
                        hsl = slice(h * 64, (h + 1) * 64)
                        for j, ki in enumerate(kis):
                            qrel = slice((qi - ki) * 128, (qi - ki + 1) * 128)
                            tpos = (0, h * 64)
                            nc.tensor.matmul(
                                o_ps[hsl, :], vp[:, ki, hsl],
                                expt[(ki, h)][:, qrel],
                                start=(j == 0), stop=(j == len(kis) - 1),
                                tile_position=tpos)
                            nc.tensor.matmul(
                                dn_ps[hsl, :], ones_sb[:, :64],
                                expt[(ki, h)][:, qrel],
                                start=(j == 0), stop=(j == len(kis) - 1),
                                tile_position=tpos)
                    rec = pb.tile([128, 128], dt.float32, tag="rec")
                    nc.vector.reciprocal(rec[:], dn_ps[:])
                    nc.vector.tensor_mul(o2T[:, qi * 128:(qi + 1) * 128],
                                         o_ps[:], rec[:])

            # ----- AllGather heads + out-proj ------------------------------
            ag_in = dram.tile([128, T], dt.float32, tag=f"agin{blk}")
            ag_out = dram.tile([NCORE * 128, T], dt.float32, tag=f"agout{blk}", addr_space="Shared")
            nc.gpsimd.dma_start(ag_in[:], o2T[:])
            nc.gpsimd.collective_compute(
                "AllGather", OP.bypass, ins=[ag_in[:].opt()],
                outs=[ag_out[:].opt()], replica_groups=RG)

            xT2 = xT  # out-proj overwrites the residual tile in place
            with tc.tile_pool(name=f"attC{blk}", bufs=2) as pc, \
                 tc.tile_pool(name=f"psC{blk}", bufs=2, space="PSUM") as psc:
                o_all = pc.tile([128, KT, T], dt.float32, tag="oall", bufs=1)
                for ko in range(KT):
                    nc.sync.dma_start(o_all[:, ko],
                                      ag_out[ko * 128:(ko + 1) * 128, :])
                wob_sb = pc.tile([128, KT], dt.float32, tag="wob")
                nc.sync.dma_start(wob_sb[:], wob_d[blk])
                for m in range(KT):
                    wo_m = pc.tile([128, KT, 128], dt.float32, tag="wom")
                    nc.sync.dma_start(
                        wo_m[:],
                        woT[blk, :, m * 128:(m + 1) * 128]
                        .rearrange("(ko p) mm -> p ko mm", p=128))
                    for nh in range(2):
                        nsl = slice(nh * 512, (nh + 1) * 512)
                        xp = psc.tile([128, 512], dt.float32, tag="xp")
                        for ko in range(KT):
                            nc.tensor.matmul(xp[:], wo_m[:, ko],
                                             o_all[:, ko, nsl],
                                             start=(ko == 0),
                                             stop=(ko == KT - 1))
                        nc.scalar.activation(xT2[:, m, nsl], xp[:],
                                             AF.Identity,
                                             bias=wob_sb[:, m:m + 1])

            qkv_out.release()

            # ---------------- MoE ------------------------------------------
            gc_row_d = dram.tile([T], dt.float32, tag=f"gc{blk}")
            hT = tc.alloc_tile_pool(name=f"hT{blk}", bufs=1)
            hT_sb = hT.tile([128, FFT, T], dt.float16, tag="hT")
            moe_keep = tc.alloc_tile_pool(name=f"moek{blk}", bufs=1)
            xn2h = moe_keep.tile([128, KT, T], dt.float16, tag="xn2h")
            gc_bc = moe_keep.tile([128, T], dt.float32, tag="gcbc")

            with tc.tile_pool(name=f"moeA{blk}", bufs=2) as pd, \
                 tc.tile_pool(name=f"psD{blk}", bufs=1, space="PSUM") as psd:
                xn2 = pd.tile([128, KT, T], dt.float32, tag="xn2", bufs=1)
                rmsnorm(xT2, pd, psd, out_f32=xn2, out_f16=xn2h)
                # router (plain fp32 matmuls)
                rt_sb = pd.tile([128, KT, E], dt.float32, tag="rt")
                nc.sync.dma_start(rt_sb[:],
                                  rtT[blk].rearrange("(k p) e -> p k e", p=128))
                rb_sb = pd.tile([128, E], dt.float32, tag="rb")
                nc.sync.dma_start(rb_sb[:], rb_d[blk])
                rl_sb = pd.tile([128, NT, E], dt.float32, tag="rl")
                for ti in range(NT):
                    rl_ps = psd.tile([128, E], dt.float32, tag="rlp")
                    for k in range(KT):
                        nc.tensor.matmul(rl_ps[:],
                                         xn2[:, k, ti * 128:(ti + 1) * 128],
                                         rt_sb[:, k, :],
                                         start=(k == 0), stop=(k == KT - 1))
                    nc.vector.tensor_add(rl_sb[:, ti, :], rl_ps[:], rb_sb[:])
                # top-2 softmax gates, batched over all token tiles
                m1 = pd.tile([128, NT, 1], dt.float32, tag="m1")
                nc.vector.tensor_reduce(m1[:], rl_sb[:], AX.X, OP.max)
                oh1 = pd.tile([128, NT, E], dt.float32, tag="oh1")
                nc.vector.tensor_tensor(oh1[:], rl_sb[:],
                                        m1[:].to_broadcast([128, NT, E]),
                                        OP.is_ge)
                mskd = pd.tile([128, NT, E], dt.float32, tag="mskd")
                nc.vector.tensor_scalar_mul(oh1[:], oh1[:], 1e30)
                nc.vector.tensor_sub(mskd[:], rl_sb[:], oh1[:])
                m2 = pd.tile([128, NT, 1], dt.float32, tag="m2")
                nc.vector.tensor_reduce(m2[:], mskd[:], AX.X, OP.max)
                sel2 = pd.tile([128, NT, E], dt.float32, tag="sel2")
                nc.vector.tensor_tensor(sel2[:], rl_sb[:],
                                        m2[:].to_broadcast([128, NT, E]),
                                        OP.is_ge)
                sub = pd.tile([128, NT, E], dt.float32, tag="sub")
                nc.vector.tensor_tensor(sub[:], rl_sb[:],
                                        m1[:].to_broadcast([128, NT, E]),
                                        OP.subtract)
                ee = pd.tile([128, NT, E], dt.float32, tag="ee")
                nc.scalar.activation(ee[:], sub[:], AF.Exp)
                nc.vector.tensor_mul(ee[:], ee[:], sel2[:])
                ssum = pd.tile([128, NT, 1], dt.float32, tag="ssum")
                nc.vector.tensor_reduce(ssum[:], ee[:], AX.X, OP.add)
                nc.vector.reciprocal(ssum[:], ssum[:])
                gts = pd.tile([128, NT, E], dt.float32, tag="gts")
                nc.vector.tensor_mul(gts[:], ee[:],
                                     ssum[:].to_broadcast([128, NT, E]))
                nc.vector.tensor_mul(gts[:], gts[:],
                                     sel_sb[:, None, :]
                                     .to_broadcast([128, NT, E]))
                gc_col = pd.tile([128, NT], dt.float32, tag="gcc")
                nc.vector.tensor_reduce(gc_col[:], gts[:], AX.X, OP.add)
                # [p, ti] -> flat [t] in DRAM -> [1, T] -> broadcast
                nc.sync.dma_start(
                    gc_row_d[:].rearrange("(ti p) -> p ti", p=128), gc_col[:])
                gc_row = pd.tile([1, T], dt.float32, tag="gcr")
                nc.sync.dma_start(gc_row[:], gc_row_d[:][None, :])
                nc.gpsimd.partition_broadcast(gc_bc[:], gc_row[:])

                # ---- expert h = silu(w1 @ xn2 + b1) (fp16) ----------------
                b1_sb = pd.tile([128, FFT], dt.float32, tag="b1")
                nc.sync.dma_start(b1_sb[:], b1_d[blk])
                for fm in range(FFT):
                    w1_sb = pd.tile([128, KT, 128], dt.float16, tag="w1s")
                    nc.sync.dma_start(
                        w1_sb[:],
                        w1T[blk, :, fm * 128:(fm + 1) * 128]
                        .rearrange("(k p) f -> p k f", p=128))
                    for th in range(2):
                        tsl = slice(th * 512, (th + 1) * 512)
                        h_ps = psd.tile([128, 512], dt.float32, tag="hps")
                        for k in range(KT):
                            nc.tensor.matmul(h_ps[:], w1_sb[:, k],
                                             xn2h[:, k, tsl],
                                             start=(k == 0),
                                             stop=(k == KT - 1))
                        nc.scalar.activation(hT_sb[:, fm, tsl], h_ps[:],
                                             AF.Silu, bias=b1_sb[:, fm:fm + 1])

            # ---- y = w2 @ h + b2, gate, AllReduce, residual ---------------
            ar_in = dram.tile([128, KT, T], dt.float32, tag=f"arin{blk}")
            ar_out = dram.tile([128, KT, T], dt.float32, tag=f"arout{blk}", addr_space="Shared")
            with tc.tile_pool(name=f"moeB{blk}", bufs=2) as pe, \
                 tc.tile_pool(name=f"psE{blk}", bufs=2, space="PSUM") as pse:
                b2_sb = pe.tile([128, KT], dt.float32, tag="b2")
                nc.sync.dma_start(b2_sb[:], b2_d[blk])
                for m in range(KT):
                    w2_sb = pe.tile([128, FFT, 128], dt.float16, tag="w2s")
                    nc.sync.dma_start(
                        w2_sb[:],
                        w2T[blk, :, m * 128:(m + 1) * 128]
                        .rearrange("(k p) dd -> p k dd", p=128))
                    for th in range(2):
                        tsl = slice(th * 512, (th + 1) * 512)
                        y_ps = pse.tile([128, 512], dt.float32, tag="yps")
                        for fk in range(FFT):
                            nc.tensor.matmul(y_ps[:], w2_sb[:, fk],
                                             hT_sb[:, fk, tsl],
                                             start=(fk == 0),
                                             stop=(fk == FFT - 1))
                        y_sb = pe.tile([128, 512], dt.float32, tag="ysb")
                        nc.scalar.activation(y_sb[:], y_ps[:], AF.Identity,
                                             bias=b2_sb[:, m:m + 1])
                        gy = pe.tile([128, 512], dt.float32, tag="gy")
                        nc.vector.tensor_mul(gy[:], y_sb[:], gc_bc[:, tsl])
                        nc.sync.dma_start(ar_in[:, m, tsl], gy[:])
            nc.gpsimd.collective_compute(
                "AllReduce", OP.add, ins=[ar_in[:].opt()],
                outs=[ar_out[:].opt()], replica_groups=RG)
            with tc.tile_pool(name=f"res{blk}", bufs=2) as pr:
                for m in range(KT):
                    ar_sb = pr.tile([128, T], dt.float32, tag="ar")
                    nc.sync.dma_start(ar_sb[:], ar_out[:, m, :])
                    nc.vector.tensor_add(xT[:, m, :], xT[:, m, :], ar_sb[:])

            moe_keep.release()
            hT.release()

        # =================== final norm + LM head =========================
        with tc.tile_pool(name="lm", bufs=1) as pl, \
             tc.tile_pool(name="lmw", bufs=2) as plw, \
             tc.tile_pool(name="psL", bufs=2, space="PSUM") as psl:
            xfh = pl.tile([128, KT, T], dt.float16, tag="xfh", bufs=1)
            rmsnorm(xT, pl, psl, out_f16=xfh)
            lm_sb = pl.tile([128, KT, VSH], dt.float16, tag="lmw", bufs=1)
            for k in range(KT):
                nc.sync.dma_start(lm_sb[:, k],
                                  lmT[k * 128:(k + 1) * 128, :])
            lmb_sb = pl.tile([128, VSH], dt.float32, tag="lmb", bufs=1)
            nc.sync.dma_start(lmb_sb[:], lmb_d)
            NCH = VSH // 500  # 8 chunks of 500
            for m in range(NT):
                for n in range(NCH):
                    vs = slice(n * 500, (n + 1) * 500)
                    l_ps = psl.tile([128, 500], dt.float32, tag="lps")
                    for k in range(KT):
                        nc.tensor.matmul(l_ps[:],
                                         xfh[:, k, m * 128:(m + 1) * 128],
                                         lm_sb[:, k, vs],
                                         start=(k == 0), stop=(k == KT - 1))
                    lo = plw.tile([128, 500], dt.float32, tag="lo")
                    nc.vector.tensor_add(lo[:], l_ps[:], lmb_sb[:, vs])
                    nc.sync.dma_start(logits[m * 128:(m + 1) * 128, vs], lo[:])

        dram.release()
        resid.release()
        gconst.release()

    nc.compile()
    return nc


# ---------------------------------------------------------------------------
# host-side input prep / sharding
# ---------------------------------------------------------------------------
def _prep_in_maps(inputs):
    f32 = np.float32
    idx = np.asarray(inputs["idx"]).astype(np.int32).reshape(T, 1)
    emb = np.ascontiguousarray(np.asarray(inputs["emb"], f32))
    rms = np.asarray(inputs["rms_scales"], f32)
    frms = np.asarray(inputs["final_rms_scale"], f32)
    in_w = np.asarray(inputs["in_proj_w"], f32)
    in_b = np.asarray(inputs["in_proj_b"], f32)
    out_w = np.asarray(inputs["out_proj_w"], f32)
    out_b = np.asarray(inputs["out_proj_b"], f32)
    rw = np.asarray(inputs["router_w"], f32)
    rb = np.asarray(inputs["router_b"], f32)
    w1 = np.asarray(inputs["w1"], f32)
    b1 = np.asarray(inputs["b1"], f32)
    w2 = np.asarray(inputs["w2"], f32)
    b2 = np.asarray(inputs["b2"], f32)
    lm_w = np.asarray(inputs["lm_w"], f32)
    lm_b = np.asarray(inputs["lm_b"], f32)

    # replicated constants
    cosT = np.stack([_make_rope_tables(rms[b])[0] for b in range(NB)])
    sinT = np.stack([_make_rope_tables(rms[b])[1] for b in range(NB)])
    common = {
        "idx_i": idx,
        "emb": emb,
        "ident": np.eye(128, dtype=f32),
        "onesm": np.ones((128, 128), f32),
        "permT": _make_perm(),
        "maskT": _make_maskT(),
        "cosT": np.ascontiguousarray(cosT),
        "sinT": np.ascontiguousarray(sinT),
        "woT": np.ascontiguousarray(out_w.transpose(0, 2, 1)),
        "wob": np.ascontiguousarray(out_b.reshape(NB, KT, 128)
                                    .transpose(0, 2, 1)),
        "rtT": np.ascontiguousarray(
            (rw * rms[:, None, :]).transpose(0, 2, 1)),
        "rb_bc": np.ascontiguousarray(
            np.broadcast_to(rb[:, None, :], (NB, 128, E))),
        "lmb_bc": None,  # per-core below
    }
    del common["lmb_bc"]

    lm_w_pad = np.concatenate([lm_w * frms[None, :],
                               np.zeros((1, D), f32)], axis=0)
    lm_b_pad = np.concatenate([lm_b, np.zeros((1,), f32)], axis=0)

    in_maps = []
    for c in range(NCORE):
        hsl = slice(c * 128, (c + 1) * 128)
        wq = in_w[:, 0 * D:1 * D, :][:, hsl, :]
        wk = in_w[:, 1 * D:2 * D, :][:, hsl, :]
        wv = in_w[:, 2 * D:3 * D, :][:, hsl, :] * rms[:, None, :]
        m = dict(common)
        m["wqT"] = np.ascontiguousarray(wq.transpose(0, 2, 1))
        m["wkT"] = np.ascontiguousarray(wk.transpose(0, 2, 1))
        m["wvT"] = np.ascontiguousarray(wv.transpose(0, 2, 1))
        m["bq"] = np.ascontiguousarray(
            in_b[:, 0 * D:1 * D][:, hsl, None])
        m["bk"] = np.ascontiguousarray(
            in_b[:, 1 * D:2 * D][:, hsl, None])
        m["bv_bc"] = np.ascontiguousarray(np.broadcast_to(
            in_b[:, 2 * D:3 * D][:, None, hsl], (NB, 128, 128)))
        sel = np.zeros((128, E), f32)
        sel[:, c] = 1.0
        m["sel_bc"] = sel
        m["w1T"] = np.ascontiguousarray(
            (w1[:, c] * rms[:, None, :]).transpose(0, 2, 1)
        ).astype(np.float16)
        m["b1c"] = np.ascontiguousarray(
            b1[:, c].reshape(NB, FFT, 128).transpose(0, 2, 1))
        m["w2T"] = np.ascontiguousarray(
            w2[:, c].transpose(0, 2, 1)).astype(np.float16)
        m["b2c"] = np.ascontiguousarray(
            b2[:, c].reshape(NB, KT, 128).transpose(0, 2, 1))
        vsl = slice(c * VSH, (c + 1) * VSH)
        m["lmT"] = np.ascontiguousarray(
            lm_w_pad[vsl].T).astype(np.float16)
        m["lmb_bc"] = np.ascontiguousarray(
            np.broadcast_to(lm_b_pad[vsl][None, :], (128, VSH)))
        in_maps.append(m)
    return in_maps


def kernel(**inputs):
    if "nc" not in _CACHE:
        _CACHE["nc"] = _build_nc()
    nc = _CACHE["nc"]
    in_maps = _prep_in_maps(inputs)
    from concourse.bass_utils import run_bass_kernel_spmd
    res = run_bass_kernel_spmd(nc, in_maps, core_ids=list(range(NCORE)),
                               **_CACHE.get("run_kwargs", {}))
    _CACHE["last_result"] = res
    parts = [res.results[c]["logits"] for c in range(NCORE)]
    full = np.concatenate(parts, axis=1)[:, :VOUT]
    return full.reshape(B, T, VOUT).astype(np.float32)


if __name__ == "__main__":
    nc = _build_nc()
    print("built OK")
